# revision 20
# baseline (speedup 1.0000x reference)
"""Trainium2 Bass kernel for FixedPointHGRNAttention.

Reference computation (B=4, T=2048, D=2048):
    x  = round(hs*256)/256
    i  = (x @ w_i) * s_i ; f = sigmoid((x @ w_f) * s_f) ; g = (x @ w_g) * s_g
    i  = (1-f)*i ; h_t = f_t*h_{t-1} + i_t  (scan over T, per channel)
    rms = h * rsqrt(mean(h^2, ch) + eps)
    o  = rms * g_norm_w * silu(g)
    out = round(((o*s_o) @ w_o.T)*256)/256

Sharding: 8 cores = 4 batches x 2 sequence halves. Each core computes its
[b, half] slice end-to-end in transposed [channel, time] layout; the scan
carry h[b, TC-1, :] crosses the half boundary via a tiny per-chunk pair
AllReduce. No other collectives needed.

All matmuls run as fp32r (fp32 with 12-bit significand, full PE rate).
The quantized activations (multiples of 1/256, |x*256| < 2^12) and the
ternary weights are exactly representable in fp32r, so the i/f/g and
o_proj products are exact.
"""
import numpy as np

import concourse.bass as bass
import concourse.mybir as mybir
import concourse.tile as tile
from concourse import bacc
from concourse.bass_utils import run_bass_kernel_spmd

AF = mybir.ActivationFunctionType
OP = mybir.AluOpType
F32 = mybir.dt.float32
F32R = mybir.dt.float32r

MAGIC = float(3 << 22)  # 1.5*2^23: float->int round-to-nearest-even trick
B, T, D = 4, 2048, 2048
TC = T // 2         # timesteps per core
NE = D // 128       # output-channel chunks
NK = D // 128       # contraction chunks
MV = 512            # moving-operand (free dim) block
NTH = TC // MV
EPS = 1e-5
FIXUP_LAG = 3       # chunks between scan emit and carry fixup emit

REPLICA_PAIRS = [[0, 1], [2, 3], [4, 5], [6, 7]]


def _build_kernel(dbg=False):
    nc = bacc.Bacc("TRN2", target_bir_lowering=False, debug=False, num_devices=8)
    xT = nc.dram_tensor("xT", [D, TC], F32, kind="ExternalInput").ap()
    wi = nc.dram_tensor("wi", [NE, NK, 128, 128], F32R, kind="ExternalInput").ap()
    wf = nc.dram_tensor("wf", [NE, NK, 128, 128], F32R, kind="ExternalInput").ap()
    wg = nc.dram_tensor("wg", [NE, NK, 128, 128], F32R, kind="ExternalInput").ap()
    wo = nc.dram_tensor("wo", [NE, NE, 128, 128], F32R, kind="ExternalInput").ap()
    si = nc.dram_tensor("si", [128, NE], F32, kind="ExternalInput").ap()
    sf = nc.dram_tensor("sf", [128, NE], F32, kind="ExternalInput").ap()
    sg = nc.dram_tensor("sg", [128, NE], F32, kind="ExternalInput").ap()
    sogn = nc.dram_tensor("sogn", [128, NE], F32, kind="ExternalInput").ap()
    cmc = nc.dram_tensor("cmc", [128, 1], F32, kind="ExternalInput").ap()
    cmu = nc.dram_tensor("cmu", [128, 1], F32, kind="ExternalInput").ap()
    outT = nc.dram_tensor("outT", [D, TC], F32, kind="ExternalOutput").ap()
    dbg_aps = None
    if dbg:
        dbg_aps = {
            n: nc.dram_tensor(n, s, F32, kind="ExternalOutput").ap()
            for n, s in [("dbg_xq", [D, TC]), ("dbg_h", [D, TC]),
                         ("dbg_f0", [128, TC]), ("dbg_ig0", [128, TC]),
                         ("dbg_hsw", [D, TC]), ("dbg_r", [1, TC]),
                         ("dbg_o", [D, TC])]}

    with tile.TileContext(nc) as tc:
        _body(tc, xT, wi, wf, wg, wo, si, sf, sg, sogn, cmc, cmu, outT, dbg_aps)
    nc.compile()
    return nc


def _body(tc, xT, wi, wf, wg, wo, si, sf, sg, sogn, cmc, cmu, outT, dbg_aps=None):
    nc = tc.nc
    from contextlib import ExitStack
    with ExitStack() as ctx:
        singles = ctx.enter_context(tc.tile_pool(name="singles", bufs=1))
        big = ctx.enter_context(tc.tile_pool(name="big", bufs=1))
        work = ctx.enter_context(tc.tile_pool(name="work", bufs=3))
        wpool = ctx.enter_context(tc.tile_pool(name="wpool", bufs=4))
        cols = ctx.enter_context(tc.tile_pool(name="cols", bufs=4))
        dram = ctx.enter_context(tc.tile_pool(name="dram", bufs=4, space="DRAM"))

        # constants / scales
        si_sb = singles.tile([128, NE], F32)
        nc.sync.dma_start(out=si_sb[:], in_=si)
        sf_sb = singles.tile([128, NE], F32)
        nc.sync.dma_start(out=sf_sb[:], in_=sf)
        sg_sb = singles.tile([128, NE], F32)
        nc.sync.dma_start(out=sg_sb[:], in_=sg)
        sogn_sb = singles.tile([128, NE], F32)
        nc.sync.dma_start(out=sogn_sb[:], in_=sogn)
        cmc_sb = singles.tile([128, 1], F32)
        nc.sync.dma_start(out=cmc_sb[:], in_=cmc)
        cmu_sb = singles.tile([128, 1], F32)
        nc.sync.dma_start(out=cmu_sb[:], in_=cmu)
        ones_f32 = singles.tile([128, 1], F32)
        nc.vector.memset(ones_f32[:], 1.0)
        ones_sb = singles.tile([128, 1], F32R)
        nc.scalar.activation(ones_sb[:], ones_f32[:], AF.Copy)
        eps_sb = singles.tile([128, 1], F32)
        nc.vector.memset(eps_sb[:], EPS)

        # persistent big buffer: h (fp32). x^T lives in its own pool, freed
        # before phase 3 allocates the fp32r o buffer in the same space.
        h_all = big.tile([128, NE, TC], F32)
        xq_pool_cm = tc.tile_pool(name="xq_pool", bufs=1)
        xq_pool = xq_pool_cm.__enter__()
        xq_all = xq_pool.tile([128, NK, TC], F32R)

        # ---- phase 0: load + quantize x^T ----
        for k in range(NK):
            xraw = work.tile([128, TC], F32, tag="wka")
            nc.sync.dma_start(out=xraw[:], in_=xT[k * 128:(k + 1) * 128, :])
            tmp = work.tile([128, TC], F32, tag="wkb")
            nc.scalar.activation(tmp[:], xraw[:], AF.Copy, bias=MAGIC, scale=256.0)
            nc.vector.tensor_scalar(xq_all[:, k, :], tmp[:], MAGIC, 1.0 / 256.0,
                                    OP.subtract, OP.mult)

        # ---- phase 1: f/i matmuls, gating, scans, carry exchange ----
        fixups = []  # (e, Fc_tile, carry_dram_tile)

        def emit_fixup(e, Fc, cc_out):
            carry = cols.tile([128, 1], F32, tag="carry")
            nc.gpsimd.dma_start(out=carry[:], in_=cc_out[:])
            nc.vector.tensor_mul(carry[:], carry[:], cmu_sb[:])
            h_e = h_all[:, e, :]
            nc.vector.scalar_tensor_tensor(h_e, Fc[:], carry[:, 0:1], h_e,
                                           OP.mult, OP.add)

        with tc.tile_pool(name="ps1", bufs=2, space="PSUM") as ps1:
            for e in range(NE):
                ps_f = ps1.tile([128, TC], F32, tag="ps_f")
                ps_i = ps1.tile([128, TC], F32, tag="ps_i")
                for k in range(NK):
                    wf_t = wpool.tile([128, 128], F32R, tag="wf")
                    nc.sync.dma_start(out=wf_t[:], in_=wf[e, k])
                    wi_t = wpool.tile([128, 128], F32R, tag="wi")
                    nc.sync.dma_start(out=wi_t[:], in_=wi[e, k])
                    for th in range(NTH):
                        nc.tensor.matmul(ps_f[:, th * MV:(th + 1) * MV], wf_t[:],
                                         xq_all[:, k, th * MV:(th + 1) * MV],
                                         start=(k == 0), stop=(k == NK - 1))
                    for th in range(NTH):
                        nc.tensor.matmul(ps_i[:, th * MV:(th + 1) * MV], wi_t[:],
                                         xq_all[:, k, th * MV:(th + 1) * MV],
                                         start=(k == 0), stop=(k == NK - 1))
                f_sb = work.tile([128, TC], F32, tag="wkb")
                nc.scalar.activation(f_sb[:], ps_f[:], AF.Sigmoid,
                                     scale=sf_sb[:, e:e + 1])
                omf = work.tile([128, TC], F32, tag="wka")
                nc.scalar.activation(omf[:], f_sb[:], AF.Copy, bias=1.0, scale=-1.0)
                ig = work.tile([128, TC], F32, tag="wkc")
                nc.vector.scalar_tensor_tensor(ig[:], ps_i[:], si_sb[:, e:e + 1],
                                               omf[:], OP.mult, OP.mult)
                if dbg_aps is not None and e == 0:
                    nc.sync.dma_start(out=dbg_aps["dbg_f0"], in_=f_sb[:])
                    nc.sync.dma_start(out=dbg_aps["dbg_ig0"], in_=ig[:])
                h_e = h_all[:, e, :]
                nc.vector.tensor_tensor_scan(h_e, f_sb[:], ig[:], 0.0,
                                             OP.mult, OP.add)
                Fc = work.tile([128, TC], F32, tag="fc")
                nc.vector.tensor_tensor_scan(Fc[:], f_sb[:], f_sb[:], 1.0,
                                             OP.mult, OP.bypass)
                # pair carry exchange for this chunk
                contrib = cols.tile([128, 1], F32, tag="contrib")
                nc.vector.tensor_mul(contrib[:], h_e[:, TC - 1:TC], cmc_sb[:])
                cc_in = dram.tile([128, 1], F32, tag="cc_in")
                nc.gpsimd.dma_start(out=cc_in[:], in_=contrib[:])
                cc_out = dram.tile([128, 1], F32, tag="cc_out")
                nc.gpsimd.collective_compute(
                    "AllReduce", OP.add, replica_groups=REPLICA_PAIRS,
                    ins=[cc_in.opt()], outs=[cc_out.opt()])
                fixups.append((e, Fc, cc_out))
                if len(fixups) > FIXUP_LAG:
                    emit_fixup(*fixups.pop(0))
            for fx in fixups:
                emit_fixup(*fx)

        if dbg_aps is not None:
            for k in range(NK):
                nc.sync.dma_start(out=dbg_aps["dbg_xq"][k * 128:(k + 1) * 128, :],
                                  in_=xq_all[:, k, :].bitcast(F32))
            for e in range(NE):
                nc.sync.dma_start(out=dbg_aps["dbg_h"][e * 128:(e + 1) * 128, :],
                                  in_=h_all[:, e, :])

        # ---- phase 2: g matmuls, silu, h^2 column-sum, o = h*sogn*sw ----
        with tc.tile_pool(name="ps2", bufs=2, space="PSUM") as ps2, \
             tc.tile_pool(name="pss", bufs=1, space="PSUM") as pss:
            ss = []
            for th in range(NTH):
                ss_th = pss.tile([1, MV], F32, tag=f"ss{th}")
                ss.append(ss_th)
            for e in range(NE):
                ps_g = ps2.tile([128, TC], F32, tag="ps_g")
                for k in range(NK):
                    wg_t = wpool.tile([128, 128], F32R, tag="wg")
                    nc.sync.dma_start(out=wg_t[:], in_=wg[e, k])
                    for th in range(NTH):
                        nc.tensor.matmul(ps_g[:, th * MV:(th + 1) * MV], wg_t[:],
                                         xq_all[:, k, th * MV:(th + 1) * MV],
                                         start=(k == 0), stop=(k == NK - 1))
                h_e = h_all[:, e, :]
                sq = work.tile([128, TC], F32R, tag="sq")
                nc.scalar.activation(sq[:], h_e, AF.Square)
                for th in range(NTH):
                    nc.tensor.matmul(ss[th][:], ones_sb[:],
                                     sq[:, th * MV:(th + 1) * MV],
                                     start=(e == 0), stop=(e == NE - 1))
                sw = work.tile([128, TC], F32, tag="wkb")
                nc.scalar.activation(sw[:], ps_g[:], AF.Silu,
                                     scale=sg_sb[:, e:e + 1])
                nc.vector.scalar_tensor_tensor(h_e, h_e, sogn_sb[:, e:e + 1],
                                               sw[:], OP.mult, OP.mult)

            # rms_inv = 1/sqrt(mean + eps), broadcast across partitions
            r_row = singles.tile([1, TC], F32)
            for th in range(NTH):
                nc.scalar.activation(r_row[:, th * MV:(th + 1) * MV], ss[th][:],
                                     AF.Sqrt, bias=eps_sb[:1, 0:1], scale=1.0 / D)
            nc.vector.reciprocal(r_row[:], r_row[:])
            r_dram = dram.tile([1, TC], F32, tag="r_dram")
            nc.sync.dma_start(out=r_dram[:], in_=r_row[:])
            R_sb = singles.tile([128, TC], F32)
            nc.sync.dma_start(out=R_sb[:], in_=r_dram[:].to_broadcast([128, TC]))

        if dbg_aps is not None:
            nc.sync.dma_start(out=dbg_aps["dbg_r"], in_=r_row[:])
            for e in range(NE):
                nc.sync.dma_start(out=dbg_aps["dbg_hsw"][e * 128:(e + 1) * 128, :],
                                  in_=h_all[:, e, :])

        # ---- phase 3: o = h*R (as fp32r), out^T = wo.T @ o, final round ----
        xq_pool_cm.__exit__(None, None, None)
        o_pool = ctx.enter_context(tc.tile_pool(name="o_pool", bufs=1))
        o_all = o_pool.tile([128, NE, TC], F32R)
        for e in range(NE):
            nc.vector.tensor_tensor(o_all[:, e, :], h_all[:, e, :], R_sb[:], OP.mult)
        if dbg_aps is not None:
            for e in range(NE):
                nc.sync.dma_start(out=dbg_aps["dbg_o"][e * 128:(e + 1) * 128, :],
                                  in_=o_all[:, e, :].bitcast(F32))
        with tc.tile_pool(name="ps3", bufs=2, space="PSUM") as ps3:
            for d in range(NE):
                ps_o = ps3.tile([128, TC], F32, tag="ps_o")
                for e in range(NE):
                    wo_t = wpool.tile([128, 128], F32R, tag="wo")
                    nc.sync.dma_start(out=wo_t[:], in_=wo[e, d])
                    for th in range(NTH):
                        nc.tensor.matmul(ps_o[:, th * MV:(th + 1) * MV], wo_t[:],
                                         o_all[:, e, th * MV:(th + 1) * MV],
                                         start=(e == 0), stop=(e == NE - 1))
                t1 = work.tile([128, TC], F32, tag="wka")
                nc.scalar.activation(t1[:], ps_o[:], AF.Copy, bias=MAGIC, scale=256.0)
                ot = work.tile([128, TC], F32, tag="wkb")
                nc.vector.tensor_scalar(ot[:], t1[:], MAGIC, 1.0 / 256.0,
                                        OP.subtract, OP.mult)
                nc.sync.dma_start(out=outT[d * 128:(d + 1) * 128, :], in_=ot[:])


_NC_CACHE = None


def _get_nc():
    global _NC_CACHE
    if _NC_CACHE is None:
        _NC_CACHE = _build_kernel()
    return _NC_CACHE


def _retile(w):
    # [D, E] -> [NE, NK, 128, 128] with tile[e, k] = w[k*128:(k+1)*128, e*128:(e+1)*128]
    return np.ascontiguousarray(
        w.reshape(NK, 128, NE, 128).transpose(2, 0, 1, 3))


def _scale_cols(s):
    # [D] -> [128, NE] with column e = s[e*128:(e+1)*128]
    return np.ascontiguousarray(s.reshape(NE, 128).T)


def _make_in_maps(inputs):
    hidden_states = np.asarray(inputs["hidden_states"], dtype=np.float32)
    wi_t = _retile(np.asarray(inputs["w_i"], np.float32))
    wf_t = _retile(np.asarray(inputs["w_f"], np.float32))
    wg_t = _retile(np.asarray(inputs["w_g"], np.float32))
    # o_proj: kernel reads wo[e, d] as lhsT = (w_o.T)[e*128:(e+1)*128, d*128:(d+1)*128]
    woT = np.asarray(inputs["w_o"], np.float32).T
    wo_t = np.ascontiguousarray(
        woT.reshape(NE, 128, NE, 128).transpose(0, 2, 1, 3))
    si_c = _scale_cols(np.asarray(inputs["s_i"], np.float32))
    sf_c = _scale_cols(np.asarray(inputs["s_f"], np.float32))
    sg_c = _scale_cols(np.asarray(inputs["s_g"], np.float32))
    sogn_c = _scale_cols(np.asarray(inputs["s_o"], np.float32)
                         * np.asarray(inputs["g_norm_w"], np.float32))

    in_maps = []
    for c in range(8):
        b, half = divmod(c, 2)
        xT = np.ascontiguousarray(
            hidden_states[b, half * TC:(half + 1) * TC, :].T)
        in_maps.append({
            "xT": xT, "wi": wi_t, "wf": wf_t, "wg": wg_t, "wo": wo_t,
            "si": si_c, "sf": sf_c, "sg": sg_c, "sogn": sogn_c,
            "cmc": np.full((128, 1), 1.0 - half, np.float32),
            "cmu": np.full((128, 1), float(half), np.float32),
        })
    return in_maps


def kernel(hidden_states, w_i, w_f, w_g, w_o, s_i, s_f, s_g, s_o, g_norm_w):
    nc = _get_nc()
    in_maps = _make_in_maps(dict(
        hidden_states=hidden_states, w_i=w_i, w_f=w_f, w_g=w_g, w_o=w_o,
        s_i=s_i, s_f=s_f, s_g=s_g, s_o=s_o, g_norm_w=g_norm_w))
    res = run_bass_kernel_spmd(nc, in_maps, list(range(8)))
    out = np.empty((B, T, D), np.float32)
    for c in range(8):
        b, half = divmod(c, 2)
        out[b, half * TC:(half + 1) * TC, :] = res.results[c]["outT"].T
    return out
